# revision 5
# baseline (speedup 1.0000x reference)
"""Trainium2 Bass kernel for a dense-transformer attention block.

Reference semantics (T=2048, D=2048, 16 heads, d_h=128):
    h = RMSNorm(x) * ln_w
    q,k,v = h @ W{q,k,v}.T  -> (n_h, T, d_h);  RoPE(q, k)
    att = softmax(causal(q k^T / sqrt(d_h))) @ v
    out = x + att @ Wo.T          (attention_mask is all-ones per spec)

Distribution: head-parallel over 8 cores (2 heads/core).

Schedule (per core) — software-pipelined over 512-row t-blocks:
    prologue   chunked input DMAs; srow(0) row-sums; r-chain(0); QKV(0)
    loop B:    rope(B) -> [phase4(B-2)] -> attention(B)+AllGather(B)
               -> srow(B+1) -> r-chain(B+1) -> QKV(B+1)
    tail       phase4(2), phase4(3)
The r-chain (RMSNorm rsqrt) runs on DVE/GpSimd in a [128,4] layout while
the PE does the next block's projections; r is folded into Q/K during the
PSUM drain (tensor_tensor with a broadcast r row) and into V via a fused
per-partition tensor_scalar.  Causal masking runs post-exp on GpSimd
(affine_select, fill=0), with diagonal J-chunks ordered first so the
selects hide under later chunks' work.  Phase 4 for blocks 0/1 fills
phase-2 bubbles; blocks 2/3 fill the final AllGather wait.
"""

import math

import numpy as np

EPS = 1e-5

CFG_FULL = dict(T=2048, D=2048, n_cores=8, heads_per_core=2)


# --------------------------------------------------------------------------
# device program
# --------------------------------------------------------------------------
def build_nc(T, D, n_cores, heads_per_core):
    import concourse.mybir as mybir
    import concourse.tile as tile
    from concourse import bacc

    DH = 128                      # head dim (hard-wired into layout)
    P = 128                       # partitions
    NH = heads_per_core
    DL = NH * DH                  # local width (q/k/v columns per core)
    KC = D // P                   # k-chunks over d_model
    TB = T // 512                 # 512-wide t blocks
    NIB = T // 512                # 512-wide i blocks
    NTS = T // P                  # 128-wide t subtiles
    f32 = mybir.dt.float32
    bf16 = mybir.dt.bfloat16
    i32 = mybir.dt.int32

    nc = bacc.Bacc("TRN2", target_bir_lowering=False, debug=False,
                   num_devices=n_cores)

    # ---- I/O ----
    xT = nc.dram_tensor("xT", [D, T], bf16, kind="ExternalInput").ap()
    xct_in = nc.dram_tensor("x_colsT", [DL, T], f32, kind="ExternalInput").ap()
    # weight tensors arrive host-pretiled in SBUF layout [P, KC*DL]
    wq_t = nc.dram_tensor("wq_t", [P, KC * DL], bf16, kind="ExternalInput").ap()
    wk_t = nc.dram_tensor("wk_t", [P, KC * DL], bf16, kind="ExternalInput").ap()
    wv_t = nc.dram_tensor("wv_t", [P, KC * DL], bf16, kind="ExternalInput").ap()
    # wo_t additionally row-permuted on host to the AllGather chunk order
    wo_t = nc.dram_tensor("wo_t", [P, KC * DL], bf16, kind="ExternalInput").ap()
    cosT = nc.dram_tensor("cosT", [DH, T], bf16, kind="ExternalInput").ap()
    sinT = nc.dram_tensor("sinT", [DH, T], bf16, kind="ExternalInput").ap()
    rot_t = nc.dram_tensor("rot_t", [DH, DH], bf16, kind="ExternalInput").ap()
    lnw = nc.dram_tensor("ln_w", [D], f32, kind="ExternalInput").ap()
    out_cT = nc.dram_tensor("out_colsT", [DL, T], f32,
                            kind="ExternalOutput").ap()

    Act = mybir.ActivationFunctionType
    Alu = mybir.AluOpType
    inv_sqrt_dh = 1.0 / math.sqrt(DH)
    MAGIC = 0x5F3759DF

    with tile.TileContext(nc) as tc, \
            tc.tile_pool(name="persist", bufs=1) as persist:
        # ---------------- long-lived tensors ----------------
        Q_sb = persist.tile([P, NH, T], bf16, tag="Q_sb")
        K_sb = persist.tile([P, NH, T], bf16, tag="K_sb")
        V_sb = persist.tile([P, NTS, DL], bf16, tag="V_sb")
        rcraw_sb = persist.tile([P, NTS], f32, tag="rcraw_sb")
        rcol_sb = persist.tile([P, NTS], f32, tag="rcol_sb")
        # raw row-sums and final r share this row: the raw values are copied
        # out to rcraw before r is written back over the same columns
        rrow_sb = persist.tile([1, T], f32, tag="rrow_sb")
        ones_bf = persist.tile([P, 1], bf16, tag="ones_bf")
        rot_sb = persist.tile([P, DH], bf16, tag="rot_sb")
        cos_sb = persist.tile([P, T], bf16, tag="cos_sb")
        sin_sb = persist.tile([P, T], bf16, tag="sin_sb")

        nc.gpsimd.dma_start(rot_sb[:], rot_t)
        nc.vector.memset(ones_bf[:], 1.0)
        warm_sb = persist.tile([P, 192], bf16, tag="warm_sb")
        nc.vector.memset(warm_sb[:], 0.0)

        with tc.tile_pool(name="dram", bufs=1, space="DRAM") as dram_pool:
            ag_shared = "Shared" if n_cores > 4 else "Local"
            ag_in = [[dram_pool.tile([DH, 512], bf16, tag=f"agi{h}_{b}",
                                     name=f"ag_in{h}_{b}")
                      for b in range(NIB)] for h in range(NH)]
            ag_out = [[dram_pool.tile([n_cores * DH, 512], bf16,
                                      addr_space=ag_shared, tag=f"ago{h}_{b}",
                                      name=f"ag_out{h}_{b}")
                       for b in range(NIB)] for h in range(NH)]

            # PE warmup: back-to-back dummy matmuls so the HAM clock gate
            # opens before the real work arrives
            with tc.tile_pool(name="warm_ps", bufs=1, space="PSUM") as wmps:
                wps = wmps.tile([P, 192], f32, tag="wm")
                for _ in range(40):
                    nc.tensor.matmul(wps[:], warm_sb[:, :128], warm_sb[:],
                                     start=True, stop=True)

            from contextlib import ExitStack
            with ExitStack() as stack:
                pool = lambda *a, **k: stack.enter_context(
                    tc.tile_pool(*a, **k))
                wpool = pool(name="wqkv", bufs=1)
                xpool = pool(name="xk", bufs=1)
                sqpool = pool(name="sq", bufs=4)
                tmppool = pool(name="tmp1", bufs=6)
                rbcpool = pool(name="rbc1", bufs=2)
                qspool = pool(name="qs1", bufs=2)
                ptpool = pool(name="pt", bufs=4)
                finpool = pool(name="fin", bufs=2)
                attpool = pool(name="attp", bufs=2)
                wopool = pool(name="wo", bufs=1)
                agpool = pool(name="ag_sb", bufs=20)
                xcpool = pool(name="xc", bufs=4)
                opool = pool(name="osb", bufs=4)
                # PSUM: 2 + 3 + 2 + 1 = 8 banks
                qkps = pool(name="qk_ps", bufs=2, space="PSUM")
                midps = pool(name="mid_ps", bufs=3, space="PSUM")
                accps = pool(name="acc_ps", bufs=2, space="PSUM")
                rowps = pool(name="row_ps", bufs=1, space="PSUM")
                lnw_sb = wpool.tile([P, KC], f32, tag="lnw")
                nc.sync.dma_start(lnw_sb[:], lnw.rearrange("(kc p) -> p kc", p=P))
                wq_sb = wpool.tile([P, KC, DL], bf16, tag="wq")
                wk_sb = wpool.tile([P, KC, DL], bf16, tag="wk")
                wv_sb = wpool.tile([P, KC, DL], bf16, tag="wv")
                wo_sb = wopool.tile([P, KC, DL], bf16, tag="wo")
                xk = [xpool.tile([P, T], bf16, tag=f"xk{kc}", name=f"xk{kc}")
                      for kc in range(KC)]
                # chunked loads: block-0 columns of every k-chunk land first
                # so the first projections can start early
                nc.sync.dma_start(wq_sb[:], wq_t.rearrange("p (kc j) -> p kc j", j=DL))
                for kc in range(KC):
                    nc.sync.dma_start(xk[kc][:, 0:512], xT[P * kc:P * (kc + 1), 0:512])
                nc.sync.dma_start(wk_sb[:], wk_t.rearrange("p (kc j) -> p kc j", j=DL))
                nc.sync.dma_start(wv_sb[:], wv_t.rearrange("p (kc j) -> p kc j", j=DL))
                for B in range(1, TB):
                    tb = slice(512 * B, 512 * B + 512)
                    for kc in range(KC):
                        eng = nc.sync if kc % 2 else nc.gpsimd
                        eng.dma_start(xk[kc][:, tb], xT[P * kc:P * (kc + 1), tb])
                nc.sync.dma_start(cos_sb[:], cosT)
                nc.sync.dma_start(sin_sb[:], sinT)
                nc.sync.dma_start(wo_sb[:], wo_t.rearrange("p (kc j) -> p kc j", j=DL))
                # fold ln_w into the projection weights (free-dim broadcast,
                # per-2-chunk so the first matmuls unblock early)
                for w in (wq_sb, wk_sb, wv_sb):
                    for q0 in range(0, KC, 2):
                        nc.vector.tensor_tensor(
                            w[:, q0:q0 + 2, :], w[:, q0:q0 + 2, :],
                            lnw_sb[:, q0:q0 + 2, None].to_broadcast(
                                (P, 2, DL)), Alu.mult)

                # ---------------- pipeline stage helpers ----------------
                def sq_srow(B):
                    """x^2 (DVE bf16) + PE row-sum accumulation -> srow psum."""
                    tb = slice(512 * B, 512 * B + 512)
                    srow = rowps.tile([1, 512], f32, tag="row",
                                      name=f"srow{B}")
                    for kc in range(KC):
                        sq = sqpool.tile([P, 512], bf16, tag="sq",
                                         name=f"sq{B}_{kc}")
                        nc.vector.tensor_tensor(sq[:], xk[kc][:, tb],
                                                xk[kc][:, tb], Alu.mult)
                        nc.tensor.matmul(srow[:], ones_bf[:], sq[:],
                                         start=(kc == 0), stop=(kc == KC - 1))
                    return srow

                def r_chain(B, srow):
                    """rsqrt(mean+eps) in [128,4] layout; emit rbc broadcast."""
                    tb = slice(512 * B, 512 * B + 512)
                    nc.scalar.copy(rrow_sb[0:1, tb], srow[:])
                    for s in range(4):
                        i = 4 * B + s
                        eng = nc.gpsimd if s % 2 else nc.sync
                        eng.dma_start(
                            out=rcraw_sb[:, i:i + 1],
                            in_=rrow_sb[0:1, 512 * B + 128 * s:
                                        512 * B + 128 * (s + 1)])
                    cs = slice(4 * B, 4 * B + 4)
                    mc = tmppool.tile([P, 4], f32, tag="mc", name=f"mc{B}")
                    nc.vector.tensor_scalar(mc[:], rcraw_sb[:, cs], 1.0 / D,
                                            EPS, Alu.mult, Alu.add)
                    ri = tmppool.tile([P, 4], i32, tag="ri", name=f"ri{B}")
                    nc.vector.tensor_scalar(ri[:], mc[:].bitcast(i32), 1, None,
                                            Alu.arith_shift_right)
                    nc.vector.tensor_scalar(ri[:], ri[:], -1, MAGIC,
                                            Alu.mult, Alu.add)
                    rv = ri[:].bitcast(f32)
                    tn = tmppool.tile([P, 4], f32, tag="tn", name=f"tn{B}")
                    for it in range(2):
                        nc.vector.tensor_tensor(tn[:], rv, rv, Alu.mult)
                        nc.vector.tensor_tensor(tn[:], tn[:], mc[:], Alu.mult)
                        nc.vector.tensor_scalar(tn[:], tn[:], -0.5, 1.5,
                                                Alu.mult, Alu.add)
                        dst = rv if it == 0 else rcol_sb[:, cs]
                        nc.vector.tensor_tensor(dst, rv, tn[:], Alu.mult)
                    for s in range(4):
                        i = 4 * B + s
                        eng = nc.gpsimd if s % 2 else nc.sync
                        eng.dma_start(
                            out=rrow_sb[0:1, 512 * B + 128 * s:
                                        512 * B + 128 * (s + 1)],
                            in_=rcol_sb[:, i:i + 1])
                    rbc = rbcpool.tile([P, 512], f32, tag="rbc",
                                       name=f"rbc{B}")
                    nc.gpsimd.partition_broadcast(rbc[:], rrow_sb[0:1, tb])
                    return rbc

                def qkv_mm(B, rbc):
                    """Q/K per-head accs (r folded in drain) + V subtiles."""
                    tb = slice(512 * B, 512 * B + 512)
                    for w_sb, dst in ((wq_sb, Q_sb), (wk_sb, K_sb)):
                        for h in range(NH):
                            hs = slice(DH * h, DH * (h + 1))
                            ps = qkps.tile([P, 512], f32, tag="qk",
                                           name=f"qk{B}_{h}")
                            for kc in range(KC):
                                nc.tensor.matmul(ps[:], w_sb[:, kc, hs],
                                                 xk[kc][:, tb],
                                                 start=(kc == 0),
                                                 stop=(kc == KC - 1))
                            nc.vector.tensor_tensor(dst[:, h, tb], ps[:],
                                                    rbc[:], Alu.mult)
                    for ts in range(4):
                        i = 4 * B + ts
                        vp = midps.tile([P, 512], f32, tag="mid",
                                        name=f"v{B}_{ts}")
                        for kc in range(KC):
                            nc.tensor.matmul(vp[:, :DL],
                                             xk[kc][:, 512 * B + P * ts:
                                                    512 * B + P * (ts + 1)],
                                             wv_sb[:, kc, :], start=(kc == 0),
                                             stop=(kc == KC - 1))
                        nc.vector.tensor_scalar(V_sb[:, i, :], vp[:, :DL],
                                                rcol_sb[:, i:i + 1], None,
                                                Alu.mult)

                def rope(B):
                    tb = slice(512 * B, 512 * B + 512)
                    for buf in (Q_sb, K_sb):
                        for h in range(NH):
                            qs = qspool.tile([P, 512], bf16, tag="qs")
                            nc.vector.tensor_tensor(qs[:], buf[:, h, tb],
                                                    sin_sb[:, tb], Alu.mult)
                            rps = midps.tile([P, 512], f32, tag="mid",
                                             name=f"rope{B}")
                            nc.tensor.matmul(rps[:], rot_sb[:], qs[:],
                                             start=True, stop=True)
                            nc.vector.tensor_tensor(buf[:, h, tb], buf[:, h, tb],
                                                    cos_sb[:, tb], Alu.mult)
                            nc.vector.tensor_tensor(buf[:, h, tb], buf[:, h, tb],
                                                    rps[:], Alu.add)

                def attention(B):
                    ib = slice(512 * B, 512 * B + 512)
                    Jmax = 4 * B + 3
                    # diagonal chunks first: their post-exp masks (GpSimd)
                    # hide under later chunks' matmuls
                    order = list(range(4 * B, 4 * B + 4)) + list(range(4 * B))
                    for h in range(NH):
                        hs = slice(DH * h, DH * (h + 1))
                        av = accps.tile([P, 512], f32, tag="acc",
                                        name=f"av{B}_{h}")
                        ssum = rowps.tile([1, 512], f32, tag="row",
                                          name=f"ss{B}_{h}")
                        for n, J in enumerate(order):
                            st = midps.tile([P, 512], f32, tag="mid",
                                            name=f"st{B}_{h}_{J}")
                            nc.tensor.matmul(st[:],
                                             K_sb[:, h, P * J:P * (J + 1)],
                                             Q_sb[:, h, ib],
                                             start=True, stop=True)
                            pt = ptpool.tile([P, 512], bf16, tag="pt",
                                             name=f"pt{B}_{h}_{J}")
                            nc.scalar.activation(pt[:], st[:], Act.Exp,
                                                 scale=inv_sqrt_dh)
                            if J >= 4 * B:
                                # keep where i - j >= 0: f - p - 128r >= 0
                                nc.gpsimd.affine_select(
                                    out=pt[:], in_=pt[:], pattern=[[1, 512]],
                                    channel_multiplier=-1,
                                    base=-128 * (J - 4 * B),
                                    compare_op=Alu.is_ge, fill=0.0)
                            nc.tensor.matmul(av[:], V_sb[:, J, hs], pt[:],
                                             start=(n == 0), stop=(n == Jmax))
                            nc.tensor.matmul(ssum[:], ones_bf[:], pt[:],
                                             start=(n == 0), stop=(n == Jmax))
                        rinv = finpool.tile([1, 512], f32, tag="rinv")
                        nc.vector.reciprocal_approx_fast(rinv[:], ssum[:])
                        rb = finpool.tile([P, 512], f32, tag="rb")
                        nc.gpsimd.partition_broadcast(rb[:], rinv[:])
                        att = attpool.tile([P, 512], bf16, tag="att")
                        nc.vector.tensor_tensor(att[:], av[:], rb[:], Alu.mult)
                        nc.sync.dma_start(ag_in[h][B][:], att[:])
                        nc.gpsimd.collective_compute(
                            "AllGather", Alu.bypass,
                            replica_groups=[list(range(n_cores))],
                            ins=[ag_in[h][B][:].opt()],
                            outs=[ag_out[h][B][:].opt()])

                def phase4(B):
                    sl = slice(512 * B, 512 * B + 512)
                    ags = []
                    for kc in range(KC):
                        h_idx, c_idx = divmod(kc, KC // NH)
                        agt = agpool.tile([P, 512], bf16, tag="ag",
                                          name=f"ag{kc}_{B}")
                        eng = nc.sync if kc % 2 == 0 else nc.gpsimd
                        eng.dma_start(
                            agt[:],
                            ag_out[h_idx][B][P * c_idx:P * (c_idx + 1), :])
                        ags.append(agt)
                    for js in range(DL // P):
                        om = accps.tile([P, 512], f32, tag="acc",
                                        name=f"om{js}_{B}")
                        for kc in range(KC):
                            nc.tensor.matmul(
                                om[:], wo_sb[:, kc, P * js:P * (js + 1)],
                                ags[kc][:], start=(kc == 0),
                                stop=(kc == KC - 1))
                        xct = xcpool.tile([P, 512], f32, tag="xct")
                        nc.sync.dma_start(xct[:], xct_in[P * js:P * (js + 1), sl])
                        osb = opool.tile([P, 512], f32, tag="osb")
                        nc.vector.tensor_tensor(osb[:], om[:], xct[:], Alu.add)
                        nc.sync.dma_start(out_cT[P * js:P * (js + 1), sl],
                                          osb[:])

                # ---------------- pipelined schedule ----------------
                srow = sq_srow(0)
                rbc = r_chain(0, srow)
                qkv_mm(0, rbc)
                for B in range(TB):
                    rope(B)
                    if B >= 2:
                        phase4(B - 2)
                    attention(B)
                    if B + 1 < TB:
                        srow = sq_srow(B + 1)
                        rbc = r_chain(B + 1, srow)
                        qkv_mm(B + 1, rbc)
                phase4(2)
                phase4(3)

    nc.compile()
    return nc


# --------------------------------------------------------------------------
# host-side prep / entry point
# --------------------------------------------------------------------------
def prepare_inputs(x, cos, sin, ln_w, Wq, Wk, Wv, Wo, n_cores, heads_per_core):
    import ml_dtypes
    bf16 = ml_dtypes.bfloat16
    DH = 128
    DL = heads_per_core * DH
    x = np.ascontiguousarray(np.asarray(x, dtype=np.float32))
    cos = np.asarray(cos, dtype=np.float32)
    sin = np.asarray(sin, dtype=np.float32)
    ln_w = np.ascontiguousarray(np.asarray(ln_w, dtype=np.float32))
    xT = np.ascontiguousarray(x.T.astype(bf16))
    cosT = np.ascontiguousarray(cos.T.astype(bf16))
    sinT = np.ascontiguousarray(sin.T.astype(bf16))
    R = np.zeros((DH, DH), dtype=np.float32)
    R[np.arange(64), np.arange(64) + 64] = -1.0
    R[np.arange(64) + 64, np.arange(64)] = 1.0
    rot_t = np.ascontiguousarray(R.T.astype(bf16))
    # AllGather chunk order: head-major, then source core; each chunk is the
    # 128 att columns (global j = DL*c' + DH*h + d) that core c' / head h sent.
    perm = np.concatenate([
        DL * cp + DH * h + np.arange(DH)
        for h in range(heads_per_core) for cp in range(n_cores)
    ])
    D = x.shape[1]
    KC = D // DH

    def pretile(wT):
        # (D, DL) -> SBUF layout [P, KC*DL]: element (p, kc, j) = wT[128 kc + p, j]
        return np.ascontiguousarray(
            wT.reshape(KC, DH, DL).transpose(1, 0, 2).reshape(DH, KC * DL)
            .astype(bf16))

    in_maps = []
    for c in range(n_cores):
        cols = slice(c * DL, (c + 1) * DL)
        woT = np.asarray(Wo, np.float32)[cols, :].T  # (D, DL)
        in_maps.append({
            "xT": xT,
            "x_colsT": np.ascontiguousarray(x[:, cols].T),
            "wq_t": pretile(np.asarray(Wq, np.float32)[cols, :].T),
            "wk_t": pretile(np.asarray(Wk, np.float32)[cols, :].T),
            "wv_t": pretile(np.asarray(Wv, np.float32)[cols, :].T),
            "wo_t": pretile(woT[perm, :]),
            "cosT": cosT,
            "sinT": sinT,
            "rot_t": rot_t,
            "ln_w": ln_w,
        })
    return in_maps


_NC_CACHE = {}


def kernel(x, cos, sin, attention_mask, ln_w, Wq, Wk, Wv, Wo,
           _trace=False, _trace_cores=None):
    from concourse.bass_utils import run_bass_kernel_spmd

    cfg = CFG_FULL
    key = tuple(sorted(cfg.items()))
    if key not in _NC_CACHE:
        _NC_CACHE[key] = build_nc(**cfg)
    nc = _NC_CACHE[key]
    n_cores = cfg["n_cores"]
    in_maps = prepare_inputs(x, cos, sin, ln_w, Wq, Wk, Wv, Wo,
                             n_cores, cfg["heads_per_core"])
    res = run_bass_kernel_spmd(nc, in_maps, core_ids=list(range(n_cores)),
                               trace=_trace, trace_cores=_trace_cores)
    out = np.concatenate(
        [res.results[c]["out_colsT"].T for c in range(n_cores)], axis=1)
    kernel.last_result = res
    return out


# revision 12
# speedup vs baseline: 1.0539x; 1.0539x over previous
"""Trainium2 Bass kernel for a dense-transformer attention block.

Reference semantics (T=2048, D=2048, 16 heads, d_h=128):
    h = RMSNorm(x) * ln_w
    q,k,v = h @ W{q,k,v}.T  -> (n_h, T, d_h);  RoPE(q, k)
    att = softmax(causal(q k^T / sqrt(d_h))) @ v
    out = x + att @ Wo.T          (attention_mask is all-ones per spec)

Distribution: head-parallel over 8 cores (2 heads/core).

Schedule (per core) — software-pipelined over 512-row t-blocks:
    prologue   chunked input DMAs; srow(0) row-sums; r-chain(0); QKV(0)
    loop B:    rope(B) -> [phase4(B-2)] -> attention(B)+AllGather(B)
               -> srow(B+1) -> r-chain(B+1) -> QKV(B+1)
    tail       phase4(2), phase4(3)
The r-chain (RMSNorm rsqrt) runs on DVE/GpSimd in a [128,4] layout while
the PE does the next block's projections; r is folded into Q/K during the
PSUM drain (tensor_tensor with a broadcast r row) and into V via a fused
per-partition tensor_scalar.  Causal masking runs post-exp on GpSimd
(affine_select, fill=0), with diagonal J-chunks ordered first so the
selects hide under later chunks' work.  Phase 4 for blocks 0/1 fills
phase-2 bubbles; blocks 2/3 fill the final AllGather wait.
"""

import math

import numpy as np

EPS = 1e-5

CFG_FULL = dict(T=2048, D=2048, n_cores=8, heads_per_core=2)


# --------------------------------------------------------------------------
# device program
# --------------------------------------------------------------------------
def build_nc(T, D, n_cores, heads_per_core):
    import concourse.mybir as mybir
    import concourse.tile as tile
    from concourse import bacc

    DH = 128                      # head dim (hard-wired into layout)
    P = 128                       # partitions
    NH = heads_per_core
    DL = NH * DH                  # local width (q/k/v columns per core)
    KC = D // P                   # k-chunks over d_model
    TB = T // 512                 # 512-wide t blocks
    NIB = T // 512                # 512-wide i blocks
    NTS = T // P                  # 128-wide t subtiles
    f32 = mybir.dt.float32
    bf16 = mybir.dt.bfloat16
    i32 = mybir.dt.int32

    nc = bacc.Bacc("TRN2", target_bir_lowering=False, debug=False,
                   num_devices=n_cores)

    # ---- I/O ----
    xT = nc.dram_tensor("xT", [D, T], bf16, kind="ExternalInput").ap()
    xct_in = nc.dram_tensor("x_colsT", [DL, T], f32, kind="ExternalInput").ap()
    # weight tensors arrive host-pretiled in SBUF layout [P, KC*DL]
    wq_t = nc.dram_tensor("wq_t", [P, KC * DL], bf16, kind="ExternalInput").ap()
    wk_t = nc.dram_tensor("wk_t", [P, KC * DL], bf16, kind="ExternalInput").ap()
    wv_t = nc.dram_tensor("wv_t", [P, KC * DL], bf16, kind="ExternalInput").ap()
    # wo_t additionally row-permuted on host to the AllGather chunk order
    wo_t = nc.dram_tensor("wo_t", [P, KC * DL], bf16, kind="ExternalInput").ap()
    cosT = nc.dram_tensor("cosT", [DH, T], bf16, kind="ExternalInput").ap()
    sinT = nc.dram_tensor("sinT", [DH, T], bf16, kind="ExternalInput").ap()
    rot_t = nc.dram_tensor("rot_t", [DH, DH], bf16, kind="ExternalInput").ap()
    lnw = nc.dram_tensor("ln_w", [D], f32, kind="ExternalInput").ap()
    out_cT = nc.dram_tensor("out_colsT", [DL, T], f32,
                            kind="ExternalOutput").ap()

    Act = mybir.ActivationFunctionType
    Alu = mybir.AluOpType
    inv_sqrt_dh = 1.0 / math.sqrt(DH)
    MAGIC = 0x5F3759DF

    with tile.TileContext(nc) as tc, \
            tc.tile_pool(name="persist", bufs=1) as persist:
        # ---------------- long-lived tensors ----------------
        Q_sb = persist.tile([P, NH, T], bf16, tag="Q_sb")
        K_sb = persist.tile([P, NH, T], bf16, tag="K_sb")
        V_sb = persist.tile([P, NTS, DL], bf16, tag="V_sb")
        rcraw_sb = persist.tile([P, NTS], f32, tag="rcraw_sb")
        rcol_sb = persist.tile([P, NTS], f32, tag="rcol_sb")
        # raw row-sums and final r share this row: the raw values are copied
        # out to rcraw before r is written back over the same columns
        rrow_sb = persist.tile([1, T], f32, tag="rrow_sb")
        ones_bf = persist.tile([P, 1], bf16, tag="ones_bf")
        # ones row: stationary for K=1 broadcast matmuls (row -> 128 parts)
        ones_row = persist.tile([1, P], f32, tag="ones_row")
        rot_sb = persist.tile([P, DH], bf16, tag="rot_sb")
        cos_sb = persist.tile([P, T], bf16, tag="cos_sb")
        sin_sb = persist.tile([P, T], bf16, tag="sin_sb")

        nc.gpsimd.dma_start(rot_sb[:], rot_t)
        nc.vector.memset(ones_bf[:], 1.0)
        nc.vector.memset(ones_row[:], 1.0)
        warm_sb = persist.tile([P, 192], bf16, tag="warm_sb")
        nc.vector.memset(warm_sb[:], 0.0)

        with tc.tile_pool(name="dram", bufs=1, space="DRAM") as dram_pool:
            ag_shared = "Shared" if n_cores > 4 else "Local"
            ag_in = [[dram_pool.tile([DH, 512], bf16, tag=f"agi{h}_{b}",
                                     name=f"ag_in{h}_{b}")
                      for b in range(NIB)] for h in range(NH)]
            ag_out = [[dram_pool.tile([n_cores * DH, 512], bf16,
                                      addr_space=ag_shared, tag=f"ago{h}_{b}",
                                      name=f"ag_out{h}_{b}")
                       for b in range(NIB)] for h in range(NH)]

            # PE warmup: back-to-back dummy matmuls so the HAM clock gate
            # opens before the real work arrives
            with tc.tile_pool(name="warm_ps", bufs=1, space="PSUM") as wmps:
                wps = wmps.tile([P, 192], f32, tag="wm")
                for _ in range(40):
                    nc.tensor.matmul(wps[:], warm_sb[:, :128], warm_sb[:],
                                     start=True, stop=True)

            from contextlib import ExitStack
            with ExitStack() as stack:
                pool = lambda *a, **k: stack.enter_context(
                    tc.tile_pool(*a, **k))
                wpool = pool(name="wqkv", bufs=1)
                xpool = pool(name="xk", bufs=1)
                sqpool = pool(name="sq", bufs=4)
                tmppool = pool(name="tmp1", bufs=6)
                rbcpool = pool(name="rbc1", bufs=2)
                qspool = pool(name="qs1", bufs=2)
                ptpool = pool(name="pt", bufs=4)
                finpool = pool(name="fin", bufs=2)
                attpool = pool(name="attp", bufs=2)
                wopool = pool(name="wo", bufs=1)
                agpool = pool(name="ag_sb", bufs=20)
                xcpool = pool(name="xc", bufs=4)
                opool = pool(name="osb", bufs=4)
                # PSUM: 2 + 3 + 2 + 1 = 8 banks
                qkps = pool(name="qk_ps", bufs=2, space="PSUM")
                midps = pool(name="mid_ps", bufs=3, space="PSUM")
                accps = pool(name="acc_ps", bufs=2, space="PSUM")
                rowps = pool(name="row_ps", bufs=1, space="PSUM")
                lnw_sb = wpool.tile([P, KC], f32, tag="lnw")
                nc.sync.dma_start(lnw_sb[:], lnw.rearrange("(kc p) -> p kc", p=P))
                wq_sb = wpool.tile([P, KC, DL], bf16, tag="wq")
                wk_sb = wpool.tile([P, KC, DL], bf16, tag="wk")
                wv_sb = wpool.tile([P, KC, DL], bf16, tag="wv")
                wo_sb = wopool.tile([P, KC, DL], bf16, tag="wo")
                xk = [xpool.tile([P, T], bf16, tag=f"xk{kc}", name=f"xk{kc}")
                      for kc in range(KC)]
                # chunked loads: block-0 columns of every k-chunk land first
                # so the first projections can start early
                nc.sync.dma_start(wq_sb[:], wq_t.rearrange("p (kc j) -> p kc j", j=DL))
                for kc in range(KC):
                    eng = nc.scalar if kc % 2 else nc.sync
                    eng.dma_start(xk[kc][:, 0:512], xT[P * kc:P * (kc + 1), 0:512])
                nc.sync.dma_start(wk_sb[:], wk_t.rearrange("p (kc j) -> p kc j", j=DL))
                nc.sync.dma_start(wv_sb[:], wv_t.rearrange("p (kc j) -> p kc j", j=DL))
                for kc in range(KC):
                    eng = nc.scalar if kc % 2 else nc.gpsimd
                    eng.dma_start(xk[kc][:, 512:T], xT[P * kc:P * (kc + 1), 512:T])
                nc.gpsimd.dma_start(cos_sb[:], cosT)
                nc.gpsimd.dma_start(sin_sb[:], sinT)
                nc.sync.dma_start(wo_sb[:], wo_t.rearrange("p (kc j) -> p kc j", j=DL))
                # fold ln_w into the projection weights (free-dim broadcast,
                # per-2-chunk so the first matmuls unblock early)
                for w in (wq_sb, wk_sb, wv_sb):
                    for q0 in range(0, KC, 2):
                        nc.vector.tensor_tensor(
                            w[:, q0:q0 + 2, :], w[:, q0:q0 + 2, :],
                            lnw_sb[:, q0:q0 + 2, None].to_broadcast(
                                (P, 2, DL)), Alu.mult)

                # ---------------- pipeline stage helpers ----------------
                def sq_srow(B):
                    """x^2 (DVE bf16) + PE row-sum accumulation -> srow psum."""
                    tb = slice(512 * B, 512 * B + 512)
                    srow = rowps.tile([1, 512], f32, tag="row",
                                      name=f"srow{B}")
                    for kc in range(KC):
                        sq = sqpool.tile([P, 512], bf16, tag="sq",
                                         name=f"sq{B}_{kc}")
                        nc.vector.tensor_tensor(sq[:], xk[kc][:, tb],
                                                xk[kc][:, tb], Alu.mult)
                        nc.tensor.matmul(srow[:], ones_bf[:], sq[:],
                                         start=(kc == 0), stop=(kc == KC - 1))
                    return srow

                def r_chain(B, srow):
                    """rsqrt(mean+eps) in [128,4] layout -> rcol + r row."""
                    nc.scalar.copy(rrow_sb[0:1, 512 * B:512 * B + 512],
                                   srow[:])
                    for s in range(4):
                        i = 4 * B + s
                        nc.sync.dma_start(
                            out=rcraw_sb[:, i:i + 1],
                            in_=rrow_sb[0:1, 512 * B + 128 * s:
                                        512 * B + 128 * (s + 1)])
                    cs = slice(4 * B, 4 * B + 4)
                    mc = tmppool.tile([P, 4], f32, tag="mc", name=f"mc{B}")
                    nc.vector.tensor_scalar(mc[:], rcraw_sb[:, cs], 1.0 / D,
                                            EPS, Alu.mult, Alu.add)
                    ri = tmppool.tile([P, 4], i32, tag="ri", name=f"ri{B}")
                    nc.vector.tensor_scalar(ri[:], mc[:].bitcast(i32), 1, None,
                                            Alu.arith_shift_right)
                    nc.vector.tensor_scalar(ri[:], ri[:], -1, MAGIC,
                                            Alu.mult, Alu.add)
                    rv = ri[:].bitcast(f32)
                    tn = tmppool.tile([P, 4], f32, tag="tn", name=f"tn{B}")
                    for it in range(2):
                        nc.vector.tensor_tensor(tn[:], rv, rv, Alu.mult)
                        nc.vector.tensor_tensor(tn[:], tn[:], mc[:], Alu.mult)
                        nc.vector.tensor_scalar(tn[:], tn[:], -0.5, 1.5,
                                                Alu.mult, Alu.add)
                        dst = rv if it == 0 else rcol_sb[:, cs]
                        nc.vector.tensor_tensor(dst, rv, tn[:], Alu.mult)
                    for s in range(4):
                        i = 4 * B + s
                        nc.sync.dma_start(
                            out=rrow_sb[0:1, 512 * B + 128 * s:
                                        512 * B + 128 * (s + 1)],
                            in_=rcol_sb[:, i:i + 1])

                def qkv_mm(B):
                    """Q/K per-head accs (r folded in drain) + V subtiles."""
                    tb = slice(512 * B, 512 * B + 512)
                    rbc = None
                    for w_sb, dst in ((wq_sb, Q_sb), (wk_sb, K_sb)):
                        for h in range(NH):
                            hs = slice(DH * h, DH * (h + 1))
                            ps = qkps.tile([P, 512], f32, tag="qk",
                                           name=f"qk{B}_{h}")
                            for kc in range(KC):
                                nc.tensor.matmul(ps[:], w_sb[:, kc, hs],
                                                 xk[kc][:, tb],
                                                 start=(kc == 0),
                                                 stop=(kc == KC - 1))
                            if rbc is None:
                                # broadcast r row to 128 partitions on the PE
                                # (K=1 matmul) then pull to SBUF on ScalarE
                                rp = midps.tile([P, 512], f32, tag="mid",
                                                name=f"rbc{B}")
                                nc.tensor.matmul(rp[:], ones_row[:],
                                                 rrow_sb[0:1, tb],
                                                 start=True, stop=True)
                                rbc = rbcpool.tile([P, 512], f32, tag="rbc",
                                                   name=f"rbc{B}")
                                nc.scalar.copy(rbc[:], rp[:])
                            nc.vector.tensor_tensor(dst[:, h, tb], ps[:],
                                                    rbc[:], Alu.mult)
                    for ts in range(4):
                        i = 4 * B + ts
                        vp = midps.tile([P, 512], f32, tag="mid",
                                        name=f"v{B}_{ts}")
                        for kc in range(KC):
                            nc.tensor.matmul(vp[:, :DL],
                                             xk[kc][:, 512 * B + P * ts:
                                                    512 * B + P * (ts + 1)],
                                             wv_sb[:, kc, :], start=(kc == 0),
                                             stop=(kc == KC - 1))
                        nc.vector.tensor_scalar(V_sb[:, i, :], vp[:, :DL],
                                                rcol_sb[:, i:i + 1], None,
                                                Alu.mult)

                def rope(B):
                    tb = slice(512 * B, 512 * B + 512)
                    for buf in (Q_sb, K_sb):
                        for h in range(NH):
                            qs = qspool.tile([P, 512], bf16, tag="qs")
                            nc.vector.tensor_tensor(qs[:], buf[:, h, tb],
                                                    sin_sb[:, tb], Alu.mult)
                            rps = midps.tile([P, 512], f32, tag="mid",
                                             name=f"rope{B}")
                            nc.tensor.matmul(rps[:], rot_sb[:], qs[:],
                                             start=True, stop=True)
                            nc.vector.tensor_tensor(buf[:, h, tb], buf[:, h, tb],
                                                    cos_sb[:, tb], Alu.mult)
                            nc.vector.tensor_tensor(buf[:, h, tb], buf[:, h, tb],
                                                    rps[:], Alu.add)

                def attention(B):
                    ib = slice(512 * B, 512 * B + 512)
                    Jmax = 4 * B + 3
                    # diagonal chunks first: their post-exp masks (GpSimd)
                    # hide under later chunks' matmuls
                    order = list(range(4 * B, 4 * B + 4)) + list(range(4 * B))
                    for h in range(NH):
                        hs = slice(DH * h, DH * (h + 1))
                        av = accps.tile([P, 512], f32, tag="acc",
                                        name=f"av{B}_{h}")
                        ssum = rowps.tile([1, 512], f32, tag="row",
                                          name=f"ss{B}_{h}")
                        for n, J in enumerate(order):
                            st = midps.tile([P, 512], f32, tag="mid",
                                            name=f"st{B}_{h}_{J}")
                            nc.tensor.matmul(st[:],
                                             K_sb[:, h, P * J:P * (J + 1)],
                                             Q_sb[:, h, ib],
                                             start=True, stop=True)
                            pt = ptpool.tile([P, 512], bf16, tag="pt",
                                             name=f"pt{B}_{h}_{J}")
                            nc.scalar.activation(pt[:], st[:], Act.Exp,
                                                 scale=inv_sqrt_dh)
                            if J >= 4 * B:
                                # keep where i - j >= 0: f - p - 128r >= 0
                                nc.gpsimd.affine_select(
                                    out=pt[:], in_=pt[:], pattern=[[1, 512]],
                                    channel_multiplier=-1,
                                    base=-128 * (J - 4 * B),
                                    compare_op=Alu.is_ge, fill=0.0)
                            nc.tensor.matmul(av[:], V_sb[:, J, hs], pt[:],
                                             start=(n == 0), stop=(n == Jmax))
                            nc.tensor.matmul(ssum[:], ones_bf[:], pt[:],
                                             start=(n == 0), stop=(n == Jmax))
                        rinv = finpool.tile([1, 512], f32, tag="rinv")
                        nc.vector.reciprocal_approx_fast(rinv[:], ssum[:])
                        rbp = midps.tile([P, 512], f32, tag="mid",
                                         name=f"rb{B}_{h}")
                        nc.tensor.matmul(rbp[:], ones_row[:], rinv[:],
                                         start=True, stop=True)
                        rb = finpool.tile([P, 512], f32, tag="rb")
                        nc.scalar.copy(rb[:], rbp[:])
                        att = attpool.tile([P, 512], bf16, tag="att")
                        nc.vector.tensor_tensor(att[:], av[:], rb[:], Alu.mult)
                        nc.sync.dma_start(ag_in[h][B][:], att[:])
                        nc.gpsimd.collective_compute(
                            "AllGather", Alu.bypass,
                            replica_groups=[list(range(n_cores))],
                            ins=[ag_in[h][B][:].opt()],
                            outs=[ag_out[h][B][:].opt()])

                def phase4(B):
                    sl = slice(512 * B, 512 * B + 512)
                    ags = []
                    for kc in range(KC):
                        h_idx, c_idx = divmod(kc, KC // NH)
                        agt = agpool.tile([P, 512], bf16, tag="ag",
                                          name=f"ag{kc}_{B}")
                        eng = nc.sync if kc % 2 == 0 else nc.gpsimd
                        eng.dma_start(
                            agt[:],
                            ag_out[h_idx][B][P * c_idx:P * (c_idx + 1), :])
                        ags.append(agt)
                    for js in range(DL // P):
                        om = accps.tile([P, 512], f32, tag="acc",
                                        name=f"om{js}_{B}")
                        for kc in range(KC):
                            nc.tensor.matmul(
                                om[:], wo_sb[:, kc, P * js:P * (js + 1)],
                                ags[kc][:], start=(kc == 0),
                                stop=(kc == KC - 1))
                        xct = xcpool.tile([P, 512], f32, tag="xct")
                        nc.scalar.dma_start(xct[:], xct_in[P * js:P * (js + 1), sl])
                        osb = opool.tile([P, 512], f32, tag="osb")
                        nc.vector.tensor_tensor(osb[:], om[:], xct[:], Alu.add)
                        nc.sync.dma_start(out_cT[P * js:P * (js + 1), sl],
                                          osb[:])

                # ---------------- pipelined schedule ----------------
                srow = sq_srow(0)
                r_chain(0, srow)
                qkv_mm(0)
                for B in range(TB):
                    rope(B)
                    if B >= 2:
                        phase4(B - 2)
                    attention(B)
                    if B + 1 < TB:
                        srow = sq_srow(B + 1)
                        r_chain(B + 1, srow)
                        qkv_mm(B + 1)
                phase4(2)
                phase4(3)

    nc.compile()
    return nc


# --------------------------------------------------------------------------
# host-side prep / entry point
# --------------------------------------------------------------------------
def prepare_inputs(x, cos, sin, ln_w, Wq, Wk, Wv, Wo, n_cores, heads_per_core):
    import ml_dtypes
    bf16 = ml_dtypes.bfloat16
    DH = 128
    DL = heads_per_core * DH
    x = np.ascontiguousarray(np.asarray(x, dtype=np.float32))
    cos = np.asarray(cos, dtype=np.float32)
    sin = np.asarray(sin, dtype=np.float32)
    ln_w = np.ascontiguousarray(np.asarray(ln_w, dtype=np.float32))
    xT = np.ascontiguousarray(x.T.astype(bf16))
    cosT = np.ascontiguousarray(cos.T.astype(bf16))
    sinT = np.ascontiguousarray(sin.T.astype(bf16))
    R = np.zeros((DH, DH), dtype=np.float32)
    R[np.arange(64), np.arange(64) + 64] = -1.0
    R[np.arange(64) + 64, np.arange(64)] = 1.0
    rot_t = np.ascontiguousarray(R.T.astype(bf16))
    # AllGather chunk order: head-major, then source core; each chunk is the
    # 128 att columns (global j = DL*c' + DH*h + d) that core c' / head h sent.
    perm = np.concatenate([
        DL * cp + DH * h + np.arange(DH)
        for h in range(heads_per_core) for cp in range(n_cores)
    ])
    D = x.shape[1]
    KC = D // DH

    def pretile(wT):
        # (D, DL) -> SBUF layout [P, KC*DL]: element (p, kc, j) = wT[128 kc + p, j]
        return np.ascontiguousarray(
            wT.reshape(KC, DH, DL).transpose(1, 0, 2).reshape(DH, KC * DL)
            .astype(bf16))

    in_maps = []
    for c in range(n_cores):
        cols = slice(c * DL, (c + 1) * DL)
        woT = np.asarray(Wo, np.float32)[cols, :].T  # (D, DL)
        in_maps.append({
            "xT": xT,
            "x_colsT": np.ascontiguousarray(x[:, cols].T),
            "wq_t": pretile(np.asarray(Wq, np.float32)[cols, :].T),
            "wk_t": pretile(np.asarray(Wk, np.float32)[cols, :].T),
            "wv_t": pretile(np.asarray(Wv, np.float32)[cols, :].T),
            "wo_t": pretile(woT[perm, :]),
            "cosT": cosT,
            "sinT": sinT,
            "rot_t": rot_t,
            "ln_w": ln_w,
        })
    return in_maps


_NC_CACHE = {}


def kernel(x, cos, sin, attention_mask, ln_w, Wq, Wk, Wv, Wo,
           _trace=False, _trace_cores=None):
    from concourse.bass_utils import run_bass_kernel_spmd

    cfg = CFG_FULL
    key = tuple(sorted(cfg.items()))
    if key not in _NC_CACHE:
        _NC_CACHE[key] = build_nc(**cfg)
    nc = _NC_CACHE[key]
    n_cores = cfg["n_cores"]
    in_maps = prepare_inputs(x, cos, sin, ln_w, Wq, Wk, Wv, Wo,
                             n_cores, cfg["heads_per_core"])
    res = run_bass_kernel_spmd(nc, in_maps, core_ids=list(range(n_cores)),
                               trace=_trace, trace_cores=_trace_cores)
    out = np.concatenate(
        [res.results[c]["out_colsT"].T for c in range(n_cores)], axis=1)
    kernel.last_result = res
    return out


# revision 18
# speedup vs baseline: 1.1812x; 1.1209x over previous
"""Trainium2 Bass kernel for a dense-transformer attention block.

Reference semantics (T=2048, D=2048, 16 heads, d_h=128):
    h = RMSNorm(x) * ln_w
    q,k,v = h @ W{q,k,v}.T  -> (n_h, T, d_h);  RoPE(q, k)
    att = softmax(causal(q k^T / sqrt(d_h))) @ v
    out = x + att @ Wo.T          (attention_mask is all-ones per spec)

Distribution: head-parallel over 8 cores (2 heads/core).

Schedule (per core) — software-pipelined over 512-row t-blocks:
    prologue   chunked input DMAs; srow(0) row-sums; r-chain(0); QKV(0)
    loop B:    rope(B) -> [phase4(B-2)] -> attention(B)+AllGather(B)
               -> srow(B+1) -> r-chain(B+1) -> QKV(B+1)
    tail       phase4(2), phase4(3)
The r-chain (RMSNorm rsqrt) runs on DVE/GpSimd in a [128,4] layout while
the PE does the next block's projections; r is folded into Q/K during the
PSUM drain (tensor_tensor with a broadcast r row) and into V via a fused
per-partition tensor_scalar.  Causal masking runs post-exp on GpSimd
(affine_select, fill=0), with diagonal J-chunks ordered first so the
selects hide under later chunks' work.  Phase 4 for blocks 0/1 fills
phase-2 bubbles; blocks 2/3 fill the final AllGather wait.
"""

import math

import numpy as np

EPS = 1e-5

CFG_FULL = dict(T=2048, D=2048, n_cores=8, heads_per_core=2)


# --------------------------------------------------------------------------
# device program
# --------------------------------------------------------------------------
def build_nc(T, D, n_cores, heads_per_core):
    import concourse.mybir as mybir
    import concourse.tile as tile
    from concourse import bacc

    DH = 128                      # head dim (hard-wired into layout)
    P = 128                       # partitions
    NH = heads_per_core
    DL = NH * DH                  # local width (q/k/v columns per core)
    KC = D // P                   # k-chunks over d_model
    TB = T // 512                 # 512-wide t blocks
    NIB = T // 512                # 512-wide i blocks
    NTS = T // P                  # 128-wide t subtiles
    f32 = mybir.dt.float32
    bf16 = mybir.dt.bfloat16
    i32 = mybir.dt.int32

    nc = bacc.Bacc("TRN2", target_bir_lowering=False, debug=False,
                   num_devices=n_cores)

    # ---- I/O ----
    xT = nc.dram_tensor("xT", [D, T], bf16, kind="ExternalInput").ap()
    xct_in = nc.dram_tensor("x_colsT", [DL, T], f32, kind="ExternalInput").ap()
    # weight tensors arrive host-pretiled in SBUF layout [P, KC*DL]
    wq_t = nc.dram_tensor("wq_t", [P, KC * DL], bf16, kind="ExternalInput").ap()
    wk_t = nc.dram_tensor("wk_t", [P, KC * DL], bf16, kind="ExternalInput").ap()
    wv_t = nc.dram_tensor("wv_t", [P, KC * DL], bf16, kind="ExternalInput").ap()
    # wo_t additionally row-permuted on host to the AllGather chunk order
    wo_t = nc.dram_tensor("wo_t", [P, KC * DL], bf16, kind="ExternalInput").ap()
    cosT = nc.dram_tensor("cosT", [DH, T], bf16, kind="ExternalInput").ap()
    sinT = nc.dram_tensor("sinT", [DH, T], bf16, kind="ExternalInput").ap()
    rot_t = nc.dram_tensor("rot_t", [DH, DH], bf16, kind="ExternalInput").ap()
    lnw = nc.dram_tensor("ln_w", [D], f32, kind="ExternalInput").ap()
    out_cT = nc.dram_tensor("out_colsT", [DL, T], f32,
                            kind="ExternalOutput").ap()

    Act = mybir.ActivationFunctionType
    Alu = mybir.AluOpType
    inv_sqrt_dh = 1.0 / math.sqrt(DH)
    MAGIC = 0x5F3759DF

    with tile.TileContext(nc) as tc, \
            tc.tile_pool(name="persist", bufs=1) as persist:
        # ---------------- long-lived tensors ----------------
        Q_sb = persist.tile([P, NH, T], bf16, tag="Q_sb")
        K_sb = persist.tile([P, NH, T], bf16, tag="K_sb")
        V_sb = persist.tile([P, NTS, DL], bf16, tag="V_sb")
        rcraw_sb = persist.tile([P, NTS], f32, tag="rcraw_sb")
        rcol_sb = persist.tile([P, NTS], f32, tag="rcol_sb")
        # raw row-sums and final r share this row: the raw values are copied
        # out to rcraw before r is written back over the same columns
        rrow_sb = persist.tile([1, T], f32, tag="rrow_sb")
        ones_bf = persist.tile([P, 1], bf16, tag="ones_bf")
        # ones row: stationary for K=1 broadcast matmuls (row -> 128 parts)
        ones_row = persist.tile([1, P], bf16, tag="ones_row")
        rcolbf_sb = persist.tile([P, NTS], bf16, tag="rcolbf_sb")
        rrowbf_sb = persist.tile([1, T], bf16, tag="rrowbf_sb")
        rot_sb = persist.tile([P, DH], bf16, tag="rot_sb")
        cos_sb = persist.tile([P, T], bf16, tag="cos_sb")
        sin_sb = persist.tile([P, T], bf16, tag="sin_sb")
        masks_sb = persist.tile([P, 4, 512], f32, tag="masks_sb")

        nc.gpsimd.dma_start(rot_sb[:], rot_t)
        nc.vector.memset(ones_bf[:], 1.0)
        nc.vector.memset(ones_row[:], 1.0)
        warm_sb = persist.tile([P, 192], bf16, tag="warm_sb")
        nc.vector.memset(warm_sb[:], 0.0)
        nc.gpsimd.memset(masks_sb[:], 0.0)
        for r in range(4):
            # keep (0) where i - j >= 0 with i = 512*B + f, j = 128*J + p,
            # offset r = J - 4*B  ->  f - p - 128 r >= 0
            nc.gpsimd.affine_select(
                out=masks_sb[:, r, :], in_=masks_sb[:, r, :],
                pattern=[[1, 512]], channel_multiplier=-1, base=-128 * r,
                compare_op=Alu.is_ge, fill=-1.0e30)

        with tc.tile_pool(name="dram", bufs=1, space="DRAM") as dram_pool:
            ag_shared = "Shared" if n_cores > 4 else "Local"
            ag_in = [[dram_pool.tile([DH, 512], bf16, tag=f"agi{h}_{b}",
                                     name=f"ag_in{h}_{b}")
                      for b in range(NIB)] for h in range(NH)]
            ag_out = [[dram_pool.tile([n_cores * DH, 512], bf16,
                                      addr_space=ag_shared, tag=f"ago{h}_{b}",
                                      name=f"ag_out{h}_{b}")
                       for b in range(NIB)] for h in range(NH)]

            # PE warmup: back-to-back dummy matmuls so the HAM clock gate
            # opens before the real work arrives
            with tc.tile_pool(name="warm_ps", bufs=1, space="PSUM") as wmps:
                wps = wmps.tile([P, 192], f32, tag="wm")
                for _ in range(40):
                    nc.tensor.matmul(wps[:], warm_sb[:, :128], warm_sb[:],
                                     start=True, stop=True)

            from contextlib import ExitStack
            with ExitStack() as stack:
                pool = lambda *a, **k: stack.enter_context(
                    tc.tile_pool(*a, **k))
                wpool = pool(name="wqkv", bufs=1)
                xpool = pool(name="xk", bufs=1)
                sqpool = pool(name="sq", bufs=4)
                tmppool = pool(name="tmp1", bufs=6)
                rbcpool = pool(name="rbc1", bufs=2)
                qspool = pool(name="qs1", bufs=2)
                ptpool = pool(name="pt", bufs=4)
                finpool = pool(name="fin", bufs=2)
                attpool = pool(name="attp", bufs=2)
                wopool = pool(name="wo", bufs=1)
                agpool = pool(name="ag_sb", bufs=20)
                xcpool = pool(name="xc", bufs=2)
                opool = pool(name="osb", bufs=2)
                # PSUM: 2 + 3 + 2 + 1 = 8 banks
                qkps = pool(name="qk_ps", bufs=2, space="PSUM")
                midps = pool(name="mid_ps", bufs=3, space="PSUM")
                accps = pool(name="acc_ps", bufs=2, space="PSUM")
                rowps = pool(name="row_ps", bufs=1, space="PSUM")
                lnw_sb = wpool.tile([P, KC], f32, tag="lnw")
                nc.sync.dma_start(lnw_sb[:], lnw.rearrange("(kc p) -> p kc", p=P))
                wq_sb = wpool.tile([P, KC, DL], bf16, tag="wq")
                wk_sb = wpool.tile([P, KC, DL], bf16, tag="wk")
                wv_sb = wpool.tile([P, KC, DL], bf16, tag="wv")
                wo_sb = wopool.tile([P, KC, DL], bf16, tag="wo")
                xk = [xpool.tile([P, T], bf16, tag=f"xk{kc}", name=f"xk{kc}")
                      for kc in range(KC)]
                # chunked loads: block-0 columns of every k-chunk land first
                # so the first projections can start early
                nc.sync.dma_start(wq_sb[:], wq_t.rearrange("p (kc j) -> p kc j", j=DL))
                for kc in range(KC):
                    eng = nc.scalar if kc % 2 else nc.sync
                    eng.dma_start(xk[kc][:, 0:512], xT[P * kc:P * (kc + 1), 0:512])
                nc.sync.dma_start(wk_sb[:], wk_t.rearrange("p (kc j) -> p kc j", j=DL))
                nc.sync.dma_start(wv_sb[:], wv_t.rearrange("p (kc j) -> p kc j", j=DL))
                for kc in range(KC):
                    eng = nc.scalar if kc % 2 else nc.gpsimd
                    eng.dma_start(xk[kc][:, 512:T], xT[P * kc:P * (kc + 1), 512:T])
                nc.gpsimd.dma_start(cos_sb[:], cosT)
                nc.gpsimd.dma_start(sin_sb[:], sinT)
                nc.sync.dma_start(wo_sb[:], wo_t.rearrange("p (kc j) -> p kc j", j=DL))
                # fold ln_w into the projection weights (free-dim broadcast,
                # per-2-chunk so the first matmuls unblock early)
                for w in (wq_sb, wk_sb, wv_sb):
                    for q0 in range(0, KC, 2):
                        nc.vector.tensor_tensor(
                            w[:, q0:q0 + 2, :], w[:, q0:q0 + 2, :],
                            lnw_sb[:, q0:q0 + 2, None].to_broadcast(
                                (P, 2, DL)), Alu.mult)

                # ---------------- pipeline stage helpers ----------------
                def sq_srow(B):
                    """x^2 (DVE bf16) + PE row-sum accumulation -> srow psum."""
                    tb = slice(512 * B, 512 * B + 512)
                    srow = rowps.tile([1, 512], f32, tag="row",
                                      name=f"srow{B}")
                    for kc in range(KC):
                        sq = sqpool.tile([P, 512], bf16, tag="sq",
                                         name=f"sq{B}_{kc}")
                        nc.vector.tensor_tensor(sq[:], xk[kc][:, tb],
                                                xk[kc][:, tb], Alu.mult)
                        nc.tensor.matmul(srow[:], ones_bf[:], sq[:],
                                         start=(kc == 0), stop=(kc == KC - 1))
                    return srow

                def r_chain(B, srow):
                    """rsqrt(mean+eps) in [128,4] layout -> rcol + r row."""
                    nc.scalar.copy(rrow_sb[0:1, 512 * B:512 * B + 512],
                                   srow[:])
                    for s in range(4):
                        i = 4 * B + s
                        nc.sync.dma_start(
                            out=rcraw_sb[:, i:i + 1],
                            in_=rrow_sb[0:1, 512 * B + 128 * s:
                                        512 * B + 128 * (s + 1)])
                    cs = slice(4 * B, 4 * B + 4)
                    mc = tmppool.tile([P, 4], f32, tag="mc", name=f"mc{B}")
                    nc.vector.tensor_scalar(mc[:], rcraw_sb[:, cs], 1.0 / D,
                                            EPS, Alu.mult, Alu.add)
                    ri = tmppool.tile([P, 4], i32, tag="ri", name=f"ri{B}")
                    nc.vector.tensor_scalar(ri[:], mc[:].bitcast(i32), 1, None,
                                            Alu.arith_shift_right)
                    nc.vector.tensor_scalar(ri[:], ri[:], -1, MAGIC,
                                            Alu.mult, Alu.add)
                    rv = ri[:].bitcast(f32)
                    tn = tmppool.tile([P, 4], f32, tag="tn", name=f"tn{B}")
                    for it in range(2):
                        nc.vector.tensor_tensor(tn[:], rv, rv, Alu.mult)
                        nc.vector.tensor_tensor(tn[:], tn[:], mc[:], Alu.mult)
                        nc.vector.tensor_scalar(tn[:], tn[:], -0.5, 1.5,
                                                Alu.mult, Alu.add)
                        dst = rv if it == 0 else rcol_sb[:, cs]
                        nc.vector.tensor_tensor(dst, rv, tn[:], Alu.mult)
                    nc.vector.tensor_copy(rcolbf_sb[:, cs], rcol_sb[:, cs])
                    for s in range(4):
                        i = 4 * B + s
                        nc.sync.dma_start(
                            out=rrowbf_sb[0:1, 512 * B + 128 * s:
                                          512 * B + 128 * (s + 1)],
                            in_=rcolbf_sb[:, i:i + 1])

                def qkv_mm(B):
                    """Q/K per-head accs (r folded in drain) + V subtiles."""
                    tb = slice(512 * B, 512 * B + 512)
                    rbc = None
                    for w_sb, dst in ((wq_sb, Q_sb), (wk_sb, K_sb)):
                        for h in range(NH):
                            hs = slice(DH * h, DH * (h + 1))
                            ps = qkps.tile([P, 512], f32, tag="qk",
                                           name=f"qk{B}_{h}")
                            for kc in range(KC):
                                nc.tensor.matmul(ps[:], w_sb[:, kc, hs],
                                                 xk[kc][:, tb],
                                                 start=(kc == 0),
                                                 stop=(kc == KC - 1))
                            if rbc is None:
                                # broadcast r row to 128 partitions on the PE
                                # (K=1 matmul) then pull to SBUF on ScalarE
                                rp = midps.tile([P, 512], f32, tag="mid",
                                                name=f"rbc{B}")
                                nc.tensor.matmul(rp[:], ones_row[:],
                                                 rrowbf_sb[0:1, tb],
                                                 start=True, stop=True)
                                rbc = rbcpool.tile([P, 512], f32, tag="rbc",
                                                   name=f"rbc{B}")
                                nc.scalar.copy(rbc[:], rp[:])
                            nc.vector.tensor_tensor(dst[:, h, tb], ps[:],
                                                    rbc[:], Alu.mult)
                    for ts in range(4):
                        i = 4 * B + ts
                        vp = midps.tile([P, 512], f32, tag="mid",
                                        name=f"v{B}_{ts}")
                        for kc in range(KC):
                            nc.tensor.matmul(vp[:, :DL],
                                             xk[kc][:, 512 * B + P * ts:
                                                    512 * B + P * (ts + 1)],
                                             wv_sb[:, kc, :], start=(kc == 0),
                                             stop=(kc == KC - 1))
                        nc.vector.tensor_scalar(V_sb[:, i, :], vp[:, :DL],
                                                rcol_sb[:, i:i + 1], None,
                                                Alu.mult)

                def rope(B):
                    tb = slice(512 * B, 512 * B + 512)
                    for buf in (Q_sb, K_sb):
                        for h in range(NH):
                            qs = qspool.tile([P, 512], bf16, tag="qs")
                            nc.vector.tensor_tensor(qs[:], buf[:, h, tb],
                                                    sin_sb[:, tb], Alu.mult)
                            rps = midps.tile([P, 512], f32, tag="mid",
                                             name=f"rope{B}")
                            nc.tensor.matmul(rps[:], rot_sb[:], qs[:],
                                             start=True, stop=True)
                            nc.vector.tensor_tensor(buf[:, h, tb], buf[:, h, tb],
                                                    cos_sb[:, tb], Alu.mult)
                            nc.vector.tensor_tensor(buf[:, h, tb], buf[:, h, tb],
                                                    rps[:], Alu.add)

                def attention(B):
                    ib = slice(512 * B, 512 * B + 512)
                    Jmax = 4 * B + 3
                    # diagonal chunks first: their post-exp masks (GpSimd)
                    # hide under later chunks' matmuls
                    order = list(range(4 * B, 4 * B + 4)) + list(range(4 * B))
                    for h in range(NH):
                        hs = slice(DH * h, DH * (h + 1))
                        av = accps.tile([P, 512], f32, tag="acc",
                                        name=f"av{B}_{h}")
                        ssum = rowps.tile([1, 512], f32, tag="row",
                                          name=f"ss{B}_{h}")
                        for n, J in enumerate(order):
                            st = midps.tile([P, 512], f32, tag="mid",
                                            name=f"st{B}_{h}_{J}")
                            nc.tensor.matmul(st[:],
                                             K_sb[:, h, P * J:P * (J + 1)],
                                             Q_sb[:, h, ib],
                                             start=True, stop=True)
                            if J >= 4 * B:
                                nc.vector.tensor_tensor(
                                    st[:], st[:], masks_sb[:, J - 4 * B, :],
                                    Alu.add)
                            pt = ptpool.tile([P, 512], bf16, tag="pt",
                                             name=f"pt{B}_{h}_{J}")
                            nc.scalar.activation(pt[:], st[:], Act.Exp,
                                                 scale=inv_sqrt_dh)
                            nc.tensor.matmul(av[:], V_sb[:, J, hs], pt[:],
                                             start=(n == 0), stop=(n == Jmax))
                            nc.tensor.matmul(ssum[:], ones_bf[:], pt[:],
                                             start=(n == 0), stop=(n == Jmax))
                        rinv = finpool.tile([1, 512], f32, tag="rinv")
                        nc.vector.reciprocal_approx_fast(rinv[:], ssum[:])
                        rinvb = finpool.tile([1, 512], bf16, tag="rinvb")
                        nc.vector.tensor_copy(rinvb[:], rinv[:])
                        rbp = midps.tile([P, 512], f32, tag="mid",
                                         name=f"rb{B}_{h}")
                        nc.tensor.matmul(rbp[:], ones_row[:], rinvb[:],
                                         start=True, stop=True)
                        rb = finpool.tile([P, 512], f32, tag="rb")
                        nc.scalar.copy(rb[:], rbp[:])
                        att = attpool.tile([P, 512], bf16, tag="att")
                        nc.vector.tensor_tensor(att[:], av[:], rb[:], Alu.mult)
                        nc.sync.dma_start(ag_in[h][B][:], att[:])
                        nc.gpsimd.collective_compute(
                            "AllGather", Alu.bypass,
                            replica_groups=[list(range(n_cores))],
                            ins=[ag_in[h][B][:].opt()],
                            outs=[ag_out[h][B][:].opt()])

                def phase4(B):
                    sl = slice(512 * B, 512 * B + 512)
                    ags = []
                    for kc in range(KC):
                        h_idx, c_idx = divmod(kc, KC // NH)
                        agt = agpool.tile([P, 512], bf16, tag="ag",
                                          name=f"ag{kc}_{B}")
                        eng = nc.sync if kc % 2 == 0 else nc.gpsimd
                        eng.dma_start(
                            agt[:],
                            ag_out[h_idx][B][P * c_idx:P * (c_idx + 1), :])
                        ags.append(agt)
                    for js in range(DL // P):
                        om = accps.tile([P, 512], f32, tag="acc",
                                        name=f"om{js}_{B}")
                        for kc in range(KC):
                            nc.tensor.matmul(
                                om[:], wo_sb[:, kc, P * js:P * (js + 1)],
                                ags[kc][:], start=(kc == 0),
                                stop=(kc == KC - 1))
                        xct = xcpool.tile([P, 512], f32, tag="xct")
                        nc.scalar.dma_start(xct[:], xct_in[P * js:P * (js + 1), sl])
                        osb = opool.tile([P, 512], f32, tag="osb")
                        nc.vector.tensor_tensor(osb[:], om[:], xct[:], Alu.add)
                        nc.sync.dma_start(out_cT[P * js:P * (js + 1), sl],
                                          osb[:])

                # ---------------- pipelined schedule ----------------
                srow = sq_srow(0)
                r_chain(0, srow)
                qkv_mm(0)
                for B in range(TB):
                    rope(B)
                    attention(B)
                    if B >= 2:
                        phase4(B - 2)
                    if B + 1 < TB:
                        srow = sq_srow(B + 1)
                        r_chain(B + 1, srow)
                        qkv_mm(B + 1)
                phase4(2)
                phase4(3)

    nc.compile()
    return nc


# --------------------------------------------------------------------------
# host-side prep / entry point
# --------------------------------------------------------------------------
def prepare_inputs(x, cos, sin, ln_w, Wq, Wk, Wv, Wo, n_cores, heads_per_core):
    import ml_dtypes
    bf16 = ml_dtypes.bfloat16
    DH = 128
    DL = heads_per_core * DH
    x = np.ascontiguousarray(np.asarray(x, dtype=np.float32))
    cos = np.asarray(cos, dtype=np.float32)
    sin = np.asarray(sin, dtype=np.float32)
    ln_w = np.ascontiguousarray(np.asarray(ln_w, dtype=np.float32))
    xT = np.ascontiguousarray(x.T.astype(bf16))
    cosT = np.ascontiguousarray(cos.T.astype(bf16))
    sinT = np.ascontiguousarray(sin.T.astype(bf16))
    R = np.zeros((DH, DH), dtype=np.float32)
    R[np.arange(64), np.arange(64) + 64] = -1.0
    R[np.arange(64) + 64, np.arange(64)] = 1.0
    rot_t = np.ascontiguousarray(R.T.astype(bf16))
    # AllGather chunk order: head-major, then source core; each chunk is the
    # 128 att columns (global j = DL*c' + DH*h + d) that core c' / head h sent.
    perm = np.concatenate([
        DL * cp + DH * h + np.arange(DH)
        for h in range(heads_per_core) for cp in range(n_cores)
    ])
    D = x.shape[1]
    KC = D // DH

    def pretile(wT):
        # (D, DL) -> SBUF layout [P, KC*DL]: element (p, kc, j) = wT[128 kc + p, j]
        return np.ascontiguousarray(
            wT.reshape(KC, DH, DL).transpose(1, 0, 2).reshape(DH, KC * DL)
            .astype(bf16))

    in_maps = []
    for c in range(n_cores):
        cols = slice(c * DL, (c + 1) * DL)
        woT = np.asarray(Wo, np.float32)[cols, :].T  # (D, DL)
        in_maps.append({
            "xT": xT,
            "x_colsT": np.ascontiguousarray(x[:, cols].T),
            "wq_t": pretile(np.asarray(Wq, np.float32)[cols, :].T),
            "wk_t": pretile(np.asarray(Wk, np.float32)[cols, :].T),
            "wv_t": pretile(np.asarray(Wv, np.float32)[cols, :].T),
            "wo_t": pretile(woT[perm, :]),
            "cosT": cosT,
            "sinT": sinT,
            "rot_t": rot_t,
            "ln_w": ln_w,
        })
    return in_maps


_NC_CACHE = {}


def kernel(x, cos, sin, attention_mask, ln_w, Wq, Wk, Wv, Wo,
           _trace=False, _trace_cores=None):
    from concourse.bass_utils import run_bass_kernel_spmd

    cfg = CFG_FULL
    key = tuple(sorted(cfg.items()))
    if key not in _NC_CACHE:
        _NC_CACHE[key] = build_nc(**cfg)
    nc = _NC_CACHE[key]
    n_cores = cfg["n_cores"]
    in_maps = prepare_inputs(x, cos, sin, ln_w, Wq, Wk, Wv, Wo,
                             n_cores, cfg["heads_per_core"])
    res = run_bass_kernel_spmd(nc, in_maps, core_ids=list(range(n_cores)),
                               trace=_trace, trace_cores=_trace_cores)
    out = np.concatenate(
        [res.results[c]["out_colsT"].T for c in range(n_cores)], axis=1)
    kernel.last_result = res
    return out


# revision 19
# speedup vs baseline: 1.1925x; 1.0095x over previous
"""Trainium2 Bass kernel for a dense-transformer attention block.

Reference semantics (T=2048, D=2048, 16 heads, d_h=128):
    h = RMSNorm(x) * ln_w
    q,k,v = h @ W{q,k,v}.T  -> (n_h, T, d_h);  RoPE(q, k)
    att = softmax(causal(q k^T / sqrt(d_h))) @ v
    out = x + att @ Wo.T          (attention_mask is all-ones per spec)

Distribution: head-parallel over 8 cores (2 heads/core).  Each core:
  phase 1  QKV projections for its heads (bf16 matmuls, contract over d_model);
           RMSNorm folded in: row scales r[t] enter via r-scaled RoPE tables
           (q,k) and per-row scaling (v); ln_w is folded into the weights.
           rotate_half runs on the PE as a constant permutation matmul.
  phase 2  per-head causal attention with scores computed TRANSPOSED
           (S^T[j,i]) so no transposes are needed anywhere; softmax row-sums
           accumulate on the PE via a ones-vector matmul; exp on ScalarE.
  phase 3  per-head AllGather of att^T rows (overlaps the other head's work)
  phase 4  output projection column-shard, weight-stationary:
           out^T[:, cols_c] rows = sum_k WoT-chunk.T @ attT-chunk  + residual
Host assembles out = concat(out_colsT.T, axis=1).
"""

import math

import numpy as np

EPS = 1e-5
NEG = -1.0e30

CFG_FULL = dict(T=2048, D=2048, n_cores=8, heads_per_core=2)


# --------------------------------------------------------------------------
# device program
# --------------------------------------------------------------------------
def build_nc(T, D, n_cores, heads_per_core):
    import concourse.mybir as mybir
    import concourse.tile as tile
    from concourse import bacc

    DH = 128                      # head dim (hard-wired into layout)
    P = 128                       # partitions
    NH = heads_per_core
    DL = NH * DH                  # local width (q/k/v columns per core)
    KC = D // P                   # k-chunks over d_model
    TB = T // 512                 # 512-wide t blocks
    NIB = T // 512                # 512-wide i blocks
    NTS = T // P                  # 128-wide t subtiles
    NPAIR = max(1, NIB // 2)      # t-block pairs (AG / phase-4 granularity)
    f32 = mybir.dt.float32
    bf16 = mybir.dt.bfloat16
    i32 = mybir.dt.int32
    f8 = mybir.dt.float8e4

    nc = bacc.Bacc("TRN2", target_bir_lowering=False, debug=False,
                   num_devices=n_cores)

    # ---- I/O ----
    xT = nc.dram_tensor("xT", [D, T], bf16, kind="ExternalInput").ap()
    xct_in = nc.dram_tensor("x_colsT", [DL, T], f32, kind="ExternalInput").ap()
    # weight tensors arrive host-pretiled in SBUF layout [P, KC*DL]
    wq_t = nc.dram_tensor("wq_t", [P, KC * DL], bf16, kind="ExternalInput").ap()
    wk_t = nc.dram_tensor("wk_t", [P, KC * DL], bf16, kind="ExternalInput").ap()
    wv_t = nc.dram_tensor("wv_t", [P, KC * DL], bf16, kind="ExternalInput").ap()
    # wo_t additionally row-permuted on host to the AllGather chunk order
    wo_t = nc.dram_tensor("wo_t", [P, KC * DL], bf16, kind="ExternalInput").ap()
    cosT = nc.dram_tensor("cosT", [DH, T], f32, kind="ExternalInput").ap()
    sinT = nc.dram_tensor("sinT", [DH, T], f32, kind="ExternalInput").ap()
    rot_t = nc.dram_tensor("rot_t", [DH, DH], bf16, kind="ExternalInput").ap()
    lnw = nc.dram_tensor("ln_w", [D], f32, kind="ExternalInput").ap()
    out_cT = nc.dram_tensor("out_colsT", [DL, T], f32,
                            kind="ExternalOutput").ap()

    Act = mybir.ActivationFunctionType
    Alu = mybir.AluOpType
    inv_sqrt_dh = 1.0 / math.sqrt(DH)
    MAGIC = 0x5F3759DF

    with tile.TileContext(nc) as tc, \
            tc.tile_pool(name="persist", bufs=1) as persist:
        # ---------------- long-lived tensors ----------------
        Q_sb = persist.tile([P, NH, T], bf16, tag="Q_sb")
        K_sb = persist.tile([P, NH, T], bf16, tag="K_sb")
        V_sb = persist.tile([P, NTS, DL], bf16, tag="V_sb")
        rcol_sb = persist.tile([P, NTS], f32, tag="rcol_sb")
        rrow_sb = persist.tile([1, T], f32, tag="rrow_sb")
        ones_bf = persist.tile([P, 1], bf16, tag="ones_bf")
        masks_sb = persist.tile([P, 4, 512], f32, tag="masks_sb")
        rot_sb = persist.tile([P, DH], bf16, tag="rot_sb")

        nc.gpsimd.dma_start(rot_sb[:], rot_t)
        nc.vector.memset(ones_bf[:], 1.0)
        warm_sb = persist.tile([P, 128], bf16, tag="warm_sb")
        nc.vector.memset(warm_sb[:], 0.0)
        nc.gpsimd.memset(masks_sb[:], 0.0)
        for r in range(4):
            # keep (0) where i - j >= 0 with i = 512*B + f, j = 128*J + p,
            # offset r = J - 4*B  ->  f - p - 128 r >= 0
            nc.gpsimd.affine_select(
                out=masks_sb[:, r, :], in_=masks_sb[:, r, :],
                pattern=[[1, 512]], channel_multiplier=-1, base=-128 * r,
                compare_op=Alu.is_ge, fill=NEG)

        with tc.tile_pool(name="dram", bufs=1, space="DRAM") as dram_pool:
            ag_shared = "Shared" if n_cores > 4 else "Local"
            ag_in = [[dram_pool.tile([DH, 512], f8, tag=f"agi{h}_{b}",
                                     name=f"ag_in{h}_{b}")
                      for b in range(NIB)] for h in range(NH)]
            ag_out = [[dram_pool.tile([n_cores * DH, 512], f8,
                                      addr_space=ag_shared, tag=f"ago{h}_{b}",
                                      name=f"ag_out{h}_{b}")
                       for b in range(NIB)] for h in range(NH)]

            # PE warmup: ~5us of back-to-back dummy matmuls so the HAM
            # clock gate opens before the real work arrives
            with tc.tile_pool(name="warm_ps", bufs=1, space="PSUM") as wmps:
                wps = wmps.tile([P, 128], f32, tag="wm")
                for _ in range(40):
                    nc.tensor.matmul(wps[:], warm_sb[:], warm_sb[:],
                                     start=True, stop=True)

            # ==== phases 1+2 interleaved per t-block: QKV projections,
            # attention, and the per-(head,block) all-gather — so the
            # collective stream starts early and hides under compute.
            with (
                tc.tile_pool(name="wqkv", bufs=1) as wpool,
                tc.tile_pool(name="cs_raw", bufs=1) as cspool,
                tc.tile_pool(name="xk", bufs=1) as xpool,
                tc.tile_pool(name="sq", bufs=4) as sqpool,
                tc.tile_pool(name="tmp1", bufs=4) as tmppool,
                tc.tile_pool(name="rbc1", bufs=2) as rbcpool,
                tc.tile_pool(name="pt", bufs=3) as ptpool,
                tc.tile_pool(name="fin", bufs=2) as finpool,
                tc.tile_pool(name="qk_ps", bufs=1, space="PSUM") as qkps,
                tc.tile_pool(name="v_ps", bufs=1, space="PSUM") as vps,
                tc.tile_pool(name="row_ps", bufs=2, space="PSUM") as rowps,
                tc.tile_pool(name="st_ps", bufs=1, space="PSUM") as stpool,
                tc.tile_pool(name="av_ps", bufs=1, space="PSUM") as avpool,
            ):
                lnw_sb = wpool.tile([P, KC], f32, tag="lnw")
                nc.sync.dma_start(lnw_sb[:], lnw.rearrange("(kc p) -> p kc", p=P))
                wq_sb = wpool.tile([P, KC, DL], bf16, tag="wq")
                wk_sb = wpool.tile([P, KC, DL], bf16, tag="wk")
                wv_sb = wpool.tile([P, KC, DL], bf16, tag="wv")
                # interleave weight / x^T loads so the first q/k matmul can
                # start as soon as wq + xk[0] have landed
                xk = [xpool.tile([P, T], bf16, tag=f"xk{kc}", name=f"xk{kc}")
                      for kc in range(KC)]
                nc.sync.dma_start(wq_sb[:], wq_t.rearrange("p (kc j) -> p kc j", j=DL))
                for kc in range(KC):
                    nc.sync.dma_start(xk[kc][:], xT[P * kc:P * (kc + 1), :])
                nc.sync.dma_start(wk_sb[:], wk_t.rearrange("p (kc j) -> p kc j", j=DL))
                nc.sync.dma_start(wv_sb[:], wv_t.rearrange("p (kc j) -> p kc j", j=DL))
                # fold ln_w into the projection weights (free-dim broadcast,
                # quarter granularity so the first matmuls unblock early)
                qn = max(1, KC // 4)
                for w in (wq_sb, wk_sb, wv_sb):
                    for q0 in range(0, KC, qn):
                        nc.vector.tensor_tensor(
                            w[:, q0:q0 + qn, :], w[:, q0:q0 + qn, :],
                            lnw_sb[:, q0:q0 + qn, None].to_broadcast(
                                (P, qn, DL)), Alu.mult)

                # cos/sin tables; r is folded in per block, in place
                cos_r = cspool.tile([P, T], f32, tag="cos")
                sin_r = cspool.tile([P, T], f32, tag="sin")
                nc.sync.dma_start(cos_r[:], cosT)
                nc.sync.dma_start(sin_r[:], sinT)

                for B in range(TB):
                    tb = slice(512 * B, 512 * B + 512)
                    # ---------- phase 1 for block B ----------
                    srow = rowps.tile([1, 512], f32, tag="row")
                    qps = qkps.tile([P, NH, 512], f32, tag="qk")
                    for kc in range(KC):
                        sq = sqpool.tile([P, 512], bf16, tag="sq")
                        nc.scalar.activation(sq[:], xk[kc][:, tb], Act.Square)
                        nc.tensor.matmul(srow[:], ones_bf[:], sq[:],
                                         start=(kc == 0), stop=(kc == KC - 1))
                        for h in range(NH):
                            hs = slice(DH * h, DH * (h + 1))
                            nc.tensor.matmul(qps[:, h, :], wq_sb[:, kc, hs],
                                             xk[kc][:, tb], start=(kc == 0),
                                             stop=(kc == KC - 1))
                    for h in range(NH):
                        nc.vector.tensor_copy(Q_sb[:, h, tb], qps[:, h, :])
                    # r = rsqrt(mean + eps): bit-trick seed + 2 Newton (DVE)
                    rr = rrow_sb[0:1, tb]
                    mrow = tmppool.tile([1, 512], f32, tag="mrow")
                    nc.vector.tensor_scalar(mrow[:], srow[:], 1.0 / D, EPS,
                                            Alu.mult, Alu.add)
                    ri = tmppool.tile([1, 512], i32, tag="ri")
                    nc.vector.tensor_scalar(ri[:], mrow[:].bitcast(i32), 1, None,
                                            Alu.arith_shift_right)
                    nc.vector.tensor_scalar(ri[:], ri[:], -1, MAGIC,
                                            Alu.mult, Alu.add)
                    rrv = ri[:].bitcast(f32)
                    tn = tmppool.tile([1, 512], f32, tag="tn")
                    nc.vector.tensor_tensor(tn[:], rrv, rrv, Alu.mult)
                    nc.vector.tensor_tensor(tn[:], tn[:], mrow[:], Alu.mult)
                    nc.vector.tensor_scalar(tn[:], tn[:], -0.5, 1.5,
                                            Alu.mult, Alu.add)
                    nc.vector.tensor_tensor(rrv, rrv, tn[:], Alu.mult)
                    nc.vector.tensor_tensor(tn[:], rrv, rrv, Alu.mult)
                    nc.vector.tensor_tensor(tn[:], tn[:], mrow[:], Alu.mult)
                    nc.vector.tensor_scalar(tn[:], tn[:], -0.5, 1.5,
                                            Alu.mult, Alu.add)
                    nc.vector.tensor_tensor(rr, rrv, tn[:], Alu.mult)
                    rbc = rbcpool.tile([P, 512], f32, tag="rbc")
                    nc.gpsimd.partition_broadcast(rbc[:], rr)
                    for s in range(4):
                        i = 4 * B + s
                        nc.gpsimd.dma_start(
                            out=rcol_sb[:, i:i + 1],
                            in_=rrow_sb[0:1, 512 * B + 128 * s:
                                        512 * B + 128 * (s + 1)])
                    nc.vector.tensor_tensor(cos_r[:, tb], cos_r[:, tb], rbc[:], Alu.mult)
                    nc.vector.tensor_tensor(sin_r[:, tb], sin_r[:, tb], rbc[:], Alu.mult)
                    # K pass (reuses the same psum slot after the Q drain)
                    kps = qkps.tile([P, NH, 512], f32, tag="qk")
                    for kc in range(KC):
                        for h in range(NH):
                            hs = slice(DH * h, DH * (h + 1))
                            nc.tensor.matmul(kps[:, h, :], wk_sb[:, kc, hs],
                                             xk[kc][:, tb], start=(kc == 0),
                                             stop=(kc == KC - 1))
                    for h in range(NH):
                        nc.vector.tensor_copy(K_sb[:, h, tb], kps[:, h, :])
                    # V pass, one 512-row tile (1 psum bank) at a time
                    for ts in range(4):
                        i = 4 * B + ts
                        vp = vps.tile([P, 512], f32, tag="v")
                        for kc in range(KC):
                            nc.tensor.matmul(vp[:, :DL],
                                             xk[kc][:, 512 * B + P * ts:
                                                    512 * B + P * (ts + 1)],
                                             wv_sb[:, kc, :], start=(kc == 0),
                                             stop=(kc == KC - 1))
                        nc.vector.tensor_copy(V_sb[:, i, :], vp[:, :DL])
                        nc.vector.tensor_scalar_mul(V_sb[:, i, :], V_sb[:, i, :],
                                                    rcol_sb[:, i:i + 1])
                    # RoPE in place on SBUF (r enters via the scaled tables)
                    for buf in (Q_sb, K_sb):
                        for h in range(NH):
                            qs = tmppool.tile([P, 512], bf16, tag="qs")
                            nc.vector.tensor_tensor(qs[:], buf[:, h, tb],
                                                    sin_r[:, tb], Alu.mult)
                            rps = vps.tile([P, 512], f32, tag="v")
                            nc.tensor.matmul(rps[:], rot_sb[:], qs[:],
                                             start=True, stop=True)
                            nc.vector.tensor_tensor(buf[:, h, tb], buf[:, h, tb],
                                                    cos_r[:, tb], Alu.mult)
                            nc.vector.tensor_tensor(buf[:, h, tb], buf[:, h, tb],
                                                    rps[:], Alu.add)
                    # ---------- phase 2 for block B (both heads) ----------
                    ib = tb
                    for h in range(NH):
                        hs = slice(DH * h, DH * (h + 1))
                        av = avpool.tile([P, 512], f32, tag="av")
                        ssum = rowps.tile([1, 512], f32, tag="row")
                        Jmax = 4 * B + 3
                        for Jp in range(0, Jmax + 1, 2):
                            st = stpool.tile([P, 2, 512], f32, tag="st")
                            pt = ptpool.tile([P, 2, 512], bf16, tag="pt")
                            for gi in range(2):
                                J = Jp + gi
                                nc.tensor.matmul(st[:, gi, :],
                                                 K_sb[:, h, P * J:P * (J + 1)],
                                                 Q_sb[:, h, ib],
                                                 start=True, stop=True)
                                if J // 4 == B:
                                    nc.vector.tensor_tensor(
                                        st[:, gi, :], st[:, gi, :],
                                        masks_sb[:, J % 4, :], Alu.add)
                            nc.scalar.activation(pt[:], st[:], Act.Exp,
                                                 scale=inv_sqrt_dh)
                            for gi in range(2):
                                J = Jp + gi
                                nc.tensor.matmul(av[:], V_sb[:, J, hs],
                                                 pt[:, gi, :], start=(J == 0),
                                                 stop=(J == Jmax))
                                nc.tensor.matmul(ssum[:], ones_bf[:],
                                                 pt[:, gi, :], start=(J == 0),
                                                 stop=(J == Jmax))
                        rinv = finpool.tile([1, 512], f32, tag="rinv")
                        nc.vector.reciprocal_approx_fast(rinv[:], ssum[:])
                        rb = finpool.tile([P, 512], f32, tag="rb")
                        nc.gpsimd.partition_broadcast(rb[:], rinv[:])
                        att = finpool.tile([P, 512], f8, tag="att")
                        nc.vector.tensor_tensor(att[:], av[:], rb[:], Alu.mult)
                        nc.sync.dma_start(ag_in[h][B][:], att[:])
                        nc.gpsimd.collective_compute(
                            "AllGather", Alu.bypass,
                            replica_groups=[list(range(n_cores))],
                            ins=[ag_in[h][B][:].opt()],
                            outs=[ag_out[h][B][:].opt()])

            # ================= phase 4: output projection =================
            with (
                tc.tile_pool(name="wo", bufs=1) as wopool,
                tc.tile_pool(name="ag_sb", bufs=20) as agpool,
                tc.tile_pool(name="xc", bufs=4) as xcpool,
                tc.tile_pool(name="osb", bufs=4) as opool,
                tc.tile_pool(name="o_ps", bufs=2, space="PSUM") as ops,
            ):
                wo_sb = wopool.tile([P, KC, DL], bf16, tag="wo")
                nc.sync.dma_start(wo_sb[:], wo_t.rearrange("p (kc j) -> p kc j", j=DL))
                for B in range(NIB):
                    sl = slice(512 * B, 512 * B + 512)
                    ags = []
                    for kc in range(KC):
                        h_idx, c_idx = divmod(kc, KC // NH)
                        agt = agpool.tile([P, 512], f8, tag="ag",
                                          name=f"ag{kc}_{B}")
                        eng = nc.sync if kc % 2 == 0 else nc.gpsimd
                        eng.dma_start(
                            agt[:],
                            ag_out[h_idx][B][P * c_idx:P * (c_idx + 1), :])
                        ags.append(agt)
                    for js in range(DL // P):
                        om = ops.tile([P, 512], f32, tag="om",
                                      name=f"om{js}_{B}")
                        for kc in range(KC):
                            nc.tensor.matmul(
                                om[:], wo_sb[:, kc, P * js:P * (js + 1)],
                                ags[kc][:], start=(kc == 0),
                                stop=(kc == KC - 1))
                        xct = xcpool.tile([P, 512], f32, tag="xct")
                        nc.sync.dma_start(xct[:], xct_in[P * js:P * (js + 1), sl])
                        osb = opool.tile([P, 512], f32, tag="osb")
                        nc.vector.tensor_tensor(osb[:], om[:], xct[:], Alu.add)
                        nc.sync.dma_start(out_cT[P * js:P * (js + 1), sl],
                                          osb[:])

    nc.compile()
    return nc


# --------------------------------------------------------------------------
# host-side prep / entry point
# --------------------------------------------------------------------------
def prepare_inputs(x, cos, sin, ln_w, Wq, Wk, Wv, Wo, n_cores, heads_per_core):
    import ml_dtypes
    bf16 = ml_dtypes.bfloat16
    DH = 128
    DL = heads_per_core * DH
    x = np.ascontiguousarray(np.asarray(x, dtype=np.float32))
    cos = np.asarray(cos, dtype=np.float32)
    sin = np.asarray(sin, dtype=np.float32)
    ln_w = np.ascontiguousarray(np.asarray(ln_w, dtype=np.float32))
    xT = np.ascontiguousarray(x.T.astype(bf16))
    cosT = np.ascontiguousarray(cos.T)
    sinT = np.ascontiguousarray(sin.T)
    R = np.zeros((DH, DH), dtype=np.float32)
    R[np.arange(64), np.arange(64) + 64] = -1.0
    R[np.arange(64) + 64, np.arange(64)] = 1.0
    rot_t = np.ascontiguousarray(R.T.astype(bf16))
    # AllGather chunk order: head-major, then source core; each chunk is the
    # 128 att columns (global j = DL*c' + DH*h + d) that core c' / head h sent.
    perm = np.concatenate([
        DL * cp + DH * h + np.arange(DH)
        for h in range(heads_per_core) for cp in range(n_cores)
    ])
    D = x.shape[1]
    KC = D // DH

    def pretile(wT):
        # (D, DL) -> SBUF layout [P, KC*DL]: element (p, kc, j) = wT[128 kc + p, j]
        return np.ascontiguousarray(
            wT.reshape(KC, DH, DL).transpose(1, 0, 2).reshape(DH, KC * DL)
            .astype(bf16))

    in_maps = []
    for c in range(n_cores):
        cols = slice(c * DL, (c + 1) * DL)
        woT = np.asarray(Wo, np.float32)[cols, :].T  # (D, DL)
        in_maps.append({
            "xT": xT,
            "x_colsT": np.ascontiguousarray(x[:, cols].T),
            "wq_t": pretile(np.asarray(Wq, np.float32)[cols, :].T),
            "wk_t": pretile(np.asarray(Wk, np.float32)[cols, :].T),
            "wv_t": pretile(np.asarray(Wv, np.float32)[cols, :].T),
            "wo_t": pretile(woT[perm, :]),
            "cosT": cosT,
            "sinT": sinT,
            "rot_t": rot_t,
            "ln_w": ln_w,
        })
    return in_maps


_NC_CACHE = {}


def kernel(x, cos, sin, attention_mask, ln_w, Wq, Wk, Wv, Wo,
           _trace=False, _trace_cores=None):
    from concourse.bass_utils import run_bass_kernel_spmd

    cfg = CFG_FULL
    key = tuple(sorted(cfg.items()))
    if key not in _NC_CACHE:
        _NC_CACHE[key] = build_nc(**cfg)
    nc = _NC_CACHE[key]
    n_cores = cfg["n_cores"]
    in_maps = prepare_inputs(x, cos, sin, ln_w, Wq, Wk, Wv, Wo,
                             n_cores, cfg["heads_per_core"])
    res = run_bass_kernel_spmd(nc, in_maps, core_ids=list(range(n_cores)),
                               trace=_trace, trace_cores=_trace_cores)
    out = np.concatenate(
        [res.results[c]["out_colsT"].T for c in range(n_cores)], axis=1)
    kernel.last_result = res
    return out

